# revision 45
# baseline (speedup 1.0000x reference)
"""Trainium2 Bass kernel for a GQA attention block (B=2, S=2048, H=2048,
16 q-heads / 8 kv-heads, head_dim=128, fp32), tensor-parallel over heads
across 8 NeuronCores.

Per-core shard (core c): q-heads {2c, 2c+1}, kv-head c; wq/wk/wv column
shards, wo row shard. x is replicated (pre-transposed on host so the
contraction dim lands on SBUF partitions). Each core emits a bf16 partial
[4096, 2048] o-proj product; the host gather for the row-parallel o-proj
is a sum over the 8 partials.

All device data is bf16 (PSUM accumulation stays f32): halves HBM traffic
and enables the DVE 16-bit fast modes. DMAs are batched into whole-tile
transfers (the SP sequencer charges ~0.6-0.9us per DMA issue, so DMA
count matters as much as bytes).

Device dataflow (per core):
  A) Q^T/K^T projections in [d, tok] layout (f32 PSUM, ap-512 matmuls);
     V projected directly in natural [tok, d] layout (ap-128 matmuls,
     no PE transposes). RMSNorm sum-of-squares via GPSIMD partition
     all-reduce (q/k norm weights folded into the RoPE tables on host);
     RoPE rotate-half as a pmat matmul; rstd applied after RoPE.
  B) Causal attention per (batch, q-tile, head, 256-q-chunk):
     scores S^T [kpos,q] + exp on ACT (no max subtraction: RMSNorm
     bounds |scores| <= sqrt(128)) + causal affine_select on the
     diagonal pair; softmax denominator via bf16 pair/quad-tree adds on
     DVE + one ones-matmul per quad (PSUM-accumulated); PV with natural
     V stationary. The o-proj for q-tile T is interleaved with the
     attention chunks of q-tile T+1 to fill PE dependency bubbles.
"""

import math
import os
import sys

import numpy as np

for _p in ("/opt/trn_rl_repo", "/root/.axon_site/_ro/trn_rl_repo"):
    if os.path.isdir(_p) and _p not in sys.path:
        sys.path.insert(0, _p)
        break

import concourse.bacc as bacc
import concourse.tile as tile
from concourse import mybir

# Route every activation we use (Exp, Ln, Copy, Identity) to the single
# "natural_log_exp_and_others" table so the act-table-load pass emits one
# LoadActFuncSet for the whole kernel instead of thrashing between the
# exp-only and ln-only tables (1283ns per swap, on the ACT critical path).
# Only the pass's view of the *other* sets is altered; ids stay positional
# and the chosen set really does contain all four funcs.
_ORIG_GAT = bacc.get_activation_tables


def _gat_one_table(arch):
    AF = mybir.ActivationFunctionType
    tabs = {k: set(v) for k, v in _ORIG_GAT(arch).items()}
    strip = {AF.Exp, AF.Ln, AF.Copy, AF.Identity}
    for k in tabs:
        if k != "natural_log_exp_and_others":
            tabs[k] = tabs[k] - strip
    return tabs


bacc.get_activation_tables = _gat_one_table
from concourse.bass_isa import ReduceOp
from concourse.bass_utils import run_bass_kernel_spmd

# Problem constants (hardcoded per contract)
B, S, HID = 2, 2048, 2048
NH, NKV, D = 16, 8, 128
NCORES = 8
HQ = NH // NCORES  # q heads per core = 2
T = B * S          # 4096 tokens
EPS = 1e-5
F32 = mybir.dt.float32
BF16 = mybir.dt.bfloat16
MDT = BF16
SCALE = 1.0 / math.sqrt(D)

KT = HID // 128      # 16 contraction tiles
TT = T // 512        # 8 token tiles of 512
QT_PER_B = S // 512  # 4 q-tiles per batch


def build_nc():
    nc = bacc.Bacc("TRN2", target_bir_lowering=False, debug=False)
    xt = nc.dram_tensor("xt", [HID, T], MDT, kind="ExternalInput").ap()
    wqkv = nc.dram_tensor("wqkv", [HID, 4 * D], MDT, kind="ExternalInput").ap()
    woc = nc.dram_tensor("woc", [HQ * D, HID], MDT, kind="ExternalInput").ap()
    pmat = nc.dram_tensor("pmat", [D, D], MDT, kind="ExternalInput").ap()
    onec = nc.dram_tensor("onec", [D, 1], MDT, kind="ExternalInput").ap()
    ctq = nc.dram_tensor("ctq", [D, S], MDT, kind="ExternalInput").ap()
    stq = nc.dram_tensor("stq", [D, S], MDT, kind="ExternalInput").ap()
    ctk = nc.dram_tensor("ctk", [D, S], MDT, kind="ExternalInput").ap()
    stk = nc.dram_tensor("stk", [D, S], MDT, kind="ExternalInput").ap()
    out = nc.dram_tensor("out", [T, HID], MDT, kind="ExternalOutput").ap()

    xt_r = xt.rearrange("(kt p) t -> p kt t", p=128)
    wqkv_r = wqkv.rearrange("(kt p) m -> p kt m", p=128)

    with tile.TileContext(nc) as tc:
        from contextlib import ExitStack

        with ExitStack() as root:
            root.enter_context(nc.allow_low_precision(
                reason="bf16 device data validated against 2e-2 rel-err gate"))
            const = root.enter_context(tc.tile_pool(name="const", bufs=1))
            ones_col = const.tile([128, 1], MDT, name="ones_col")
            nc.scalar.dma_start(out=ones_col, in_=onec)
            pmat_sb = const.tile([D, D], MDT, name="pmat_sb")
            nc.scalar.dma_start(out=pmat_sb, in_=pmat)
            eps_col = const.tile([128, 1], F32, name="eps_col")
            nc.vector.memset(eps_col, EPS)

            res = root.enter_context(tc.tile_pool(name="res", bufs=1))
            wo_sb = res.tile([128, HQ, HID], MDT, name="wo_sb")
            qt_sb = res.tile([128, HQ, T], MDT, name="qt_sb")   # [d, h, tok]
            kt_sb = res.tile([128, T], MDT, name="kt_sb")       # [d, tok]
            v_sb = res.tile([128, T // 128, D], MDT, name="v_sb")  # [tok%128, chunk, d]
            tabs = {}
            for nm in ("cq", "sq", "ck", "sk"):
                tabs[nm] = res.tile([128, S], MDT, name="tab_" + nm)

            # ---------------- Phase A: QKV^T, norm, rope ----------------------
            with ExitStack() as pa:
                wqp = pa.enter_context(tc.tile_pool(name="wqp", bufs=1))
                xp = pa.enter_context(tc.tile_pool(name="xp", bufs=4))
                wp = pa.enter_context(tc.tile_pool(name="wp", bufs=2))
                psQ = pa.enter_context(tc.tile_pool(name="psQ", bufs=2, space="PSUM"))
                psK = pa.enter_context(tc.tile_pool(name="psK", bufs=2, space="PSUM"))
                psV = pa.enter_context(tc.tile_pool(name="psV", bufs=1, space="PSUM"))
                psR = pa.enter_context(tc.tile_pool(name="psR", bufs=1, space="PSUM"))

                wqkv_sb = wqp.tile([128, KT, 4 * D], MDT, name="wqkv_sb")

                tab_srcs = {"cq": ctq, "sq": stq, "ck": ctk, "sk": stk}
                for ti, t in enumerate((0, 4, 1, 5, 2, 6, 3, 7)):
                    xk = xp.tile([128, KT, 512], MDT, name="xk", tag="xk")
                    if ti == 0:
                        # interleave wqkv / x sub-DMAs so the first matmul can
                        # start after ~1.5us instead of waiting for full tiles
                        k0 = 0
                        for g, kw in enumerate((2, 2, 4, 4, 4)):
                            nc.sync.dma_start(
                                out=wqkv_sb[:, k0:k0 + kw, :],
                                in_=wqkv_r[:, k0:k0 + kw, :],
                            )
                            nc.sync.dma_start(
                                out=xk[:, k0:k0 + kw, :],
                                in_=xt_r[:, k0:k0 + kw, t * 512:(t + 1) * 512],
                            )
                            k0 += kw
                        # first table slices (s0=0): cheap, needed by ~15us
                        for nm in ("cq", "sq", "ck", "sk"):
                            nc.sync.dma_start(
                                out=tabs[nm][:, 0:512], in_=tab_srcs[nm][:, 0:512]
                            )
                    elif ti == 1:
                        # halves so tile-1 matmuls are not gated on one big DMA
                        for kh in range(2):
                            nc.sync.dma_start(
                                out=xk[:, 8 * kh:8 * (kh + 1), :],
                                in_=xt_r[:, 8 * kh:8 * (kh + 1), t * 512:(t + 1) * 512],
                            )
                    else:
                        nc.sync.dma_start(
                            out=xk, in_=xt_r[:, :, t * 512:(t + 1) * 512]
                        )
                        if ti == 2:  # table remainders
                            for nm in ("cq", "sq", "ck", "sk"):
                                nc.sync.dma_start(
                                    out=tabs[nm][:, 512:S], in_=tab_srcs[nm][:, 512:S]
                                )
                        if ti == 2:  # wo not needed until phase B
                            nc.sync.dma_start(
                                out=wo_sb, in_=woc.rearrange("(h p) n -> p h n", p=128)
                            )

                    # Q^T projections: [d, tok] layout, ap-512
                    q01 = psQ.tile([128, 2, 512], F32, name="q01", tag="q01")
                    for k in range(KT):
                        for m in range(2):
                            nc.tensor.matmul(
                                q01[:, m, :],
                                lhsT=(wqkv_sb[:, k, m * 128:(m + 1) * 128]),
                                rhs=(xk[:, k, :]),
                                start=(k == 0), stop=(k == KT - 1),
                            )
                    kps = psK.tile([128, 512], F32, name="kps", tag="kps")
                    for k in range(KT):
                        nc.tensor.matmul(
                            kps,
                            lhsT=(wqkv_sb[:, k, 2 * 128:3 * 128]),
                            rhs=(xk[:, k, :]),
                            start=(k == 0), stop=(k == KT - 1),
                        )
                    # V directly in natural [tok, d] layout, ap-128
                    vps = psV.tile([128, 4, 128], F32, name="vps", tag="vps")
                    for c in range(4):
                        for k in range(KT):
                            nc.tensor.matmul(
                                vps[:, c, :],
                                lhsT=(xk[:, k, c * 128:(c + 1) * 128]),
                                rhs=(wqkv_sb[:, k, 3 * 128:4 * 128]),
                                start=(k == 0), stop=(k == KT - 1),
                            )
                    nc.scalar.copy(v_sb[:, t * 4:(t + 1) * 4, :], vps)

                    s0 = (t % QT_PER_B) * 512  # position-in-sequence of this tile
                    for m, cosT, sinT in (
                        (0, tabs["cq"], tabs["sq"]),
                        (1, tabs["cq"], tabs["sq"]),
                        (2, tabs["ck"], tabs["sk"]),
                    ):
                        src = q01[:, m, :] if m < 2 else kps
                        qk = wp.tile([128, 512], MDT, name="qk", tag="qk")
                        nc.scalar.copy(qk, src)  # sole PSUM reader (ACT)
                        sq = wp.tile([128, 512], MDT, name="sq", tag="sq")
                        nc.vector.tensor_mul(sq, qk, qk)
                        nc.gpsimd.partition_all_reduce(sq, sq, 128, ReduceOp.add)
                        # rstd = (ssq/D+eps)^-0.5 as exp(-0.5*ln(.)): keeps every
                        # ACT func in one table (exp+ln+copy) => no
                        # LoadActFuncSet thrash when the scheduler interleaves
                        # phase-A chains with phase-B exps. The all-reduce
                        # already wrote ssq to every partition, so compute on
                        # the full tile and skip the partition broadcast.
                        rln = wp.tile([128, 512], F32, name="rln", tag="rln")
                        nc.scalar.activation(
                            rln, sq, mybir.ActivationFunctionType.Ln,
                            bias=eps_col, scale=1.0 / D,
                        )
                        rstd = wp.tile([128, 512], MDT, name="rstd", tag="rstd")
                        nc.scalar.activation(
                            rstd, rln, mybir.ActivationFunctionType.Exp, scale=-0.5,
                        )
                        shf = psR.tile([128, 512], F32, name="shf", tag="shf")
                        nc.tensor.matmul(shf, lhsT=pmat_sb, rhs=qk, start=True, stop=True)
                        t0 = wp.tile([128, 512], MDT, name="t0", tag="t0")
                        nc.vector.tensor_mul(t0, qk, cosT[:, s0:s0 + 512])
                        t1 = wp.tile([128, 512], MDT, name="t1", tag="t1")
                        nc.vector.tensor_mul(t1, shf, sinT[:, s0:s0 + 512])
                        tr = wp.tile([128, 512], MDT, name="tr", tag="tr")
                        nc.vector.tensor_add(tr, t0, t1)
                        if m < 2:
                            dst = qt_sb[:, m, t * 512:(t + 1) * 512]
                        else:
                            dst = kt_sb[:, t * 512:(t + 1) * 512]
                        nc.vector.tensor_mul(dst, tr, rstd)

            # ---------------- Phase B: causal attention + o-proj --------------
            with ExitStack() as pb:
                ep = pb.enter_context(tc.tile_pool(name="ep", bufs=12))
                wp2 = pb.enter_context(tc.tile_pool(name="wp2", bufs=4))
                sump = pb.enter_context(tc.tile_pool(name="sump", bufs=14))
                atp = pb.enter_context(tc.tile_pool(name="atp", bufs=10))
                op = pb.enter_context(tc.tile_pool(name="op", bufs=4))
                psS = pb.enter_context(tc.tile_pool(name="psS", bufs=2, space="PSUM"))
                psOD = pb.enter_context(tc.tile_pool(name="psOD", bufs=2, space="PSUM"))
                psP = pb.enter_context(tc.tile_pool(name="psP", bufs=2, space="PSUM"))

                def emit_scores(b, qt, h, qh):
                    """Sub-phase 1: score matmuls + exp + causal mask.
                    Exp is packed 4 k-tiles per instruction (amortizes the ACT
                    access latency; ACT paces phase B). Returns (etps, n_kt):
                    etps[i][:, j, :] holds k-tile 4*i+j."""
                    q0 = qt * 512
                    qq0 = q0 + qh * 256
                    n_kt = (qq0 + 256) // 128
                    etps = []
                    qsums = []
                    for kq in range((n_kt + 3) // 4):
                        nj = min(4, n_kt - 4 * kq)
                        st = psS.tile([128, 4, 256], F32, name="st", tag="st")
                        for j in range(nj):
                            kt = 4 * kq + j
                            nc.tensor.matmul(
                                st[:, j, :],
                                lhsT=(kt_sb[:, b * S + kt * 128: b * S + (kt + 1) * 128]),
                                rhs=(qt_sb[:, h, b * S + qq0: b * S + qq0 + 256]),
                                start=True, stop=True,
                            )
                        etp = ep.tile([128, 4, 256], MDT, name="et", tag="et")
                        nc.scalar.activation(
                            etp[:, 0:nj, :], st[:, 0:nj, :],
                            mybir.ActivationFunctionType.Exp, scale=SCALE,
                        )
                        if 4 * kq + nj == n_kt:  # diagonal pair: causal mask
                            for kt in (n_kt - 2, n_kt - 1):
                                nc.gpsimd.affine_select(
                                    out=etp[:, kt % 4, :], in_=etp[:, kt % 4, :],
                                    pattern=[[1, 256]],
                                    channel_multiplier=-1,
                                    base=-(kt * 128 - qq0),
                                    compare_op=mybir.AluOpType.is_ge,
                                    fill=0.0,
                                )
                        etps.append(etp)
                    return etps, qsums, n_kt

                def emit_denpv(b, qt, h, qh, etps, qsums, n_kt):
                    """Sub-phase 2: den tree + PV accumulation + at scaling."""
                    def et(kt):
                        return etps[kt // 4][:, kt % 4, :]

                    # streaming binary-counter reduction: carry[l] holds the
                    # partial sum of 2^(l+1) k-tiles; <=3 live tiles at a time
                    n_pair = n_kt // 2
                    carry = {}
                    for kp in range(n_pair):
                        cur = sump.tile([128, 256], MDT, name="ts", tag="ts")
                        nc.vector.tensor_add(cur, et(2 * kp), et(2 * kp + 1))
                        l = 0
                        while l in carry:
                            nxt = sump.tile([128, 256], MDT, name="ts", tag="ts")
                            nc.vector.tensor_add(nxt, carry.pop(l), cur)
                            cur = nxt
                            l += 1
                        carry[l] = cur
                    lvls = sorted(carry)
                    cur = carry[lvls[0]]
                    for l in lvls[1:]:
                        nxt = sump.tile([128, 256], MDT, name="ts", tag="ts")
                        nc.vector.tensor_add(nxt, carry[l], cur)
                        cur = nxt
                    level = [cur]
                    od = psOD.tile([128, 2, 256], F32, name="od", tag="od")
                    ot = od[:, 0, :]
                    den = od[0:1, 1, :]
                    # PV first (needs only the exp tiles); the den matmul's
                    # quad partials were computed during the scores pipeline,
                    # so it issues right at PV end with no tree wait
                    for kt in range(n_kt):
                        nc.tensor.matmul(
                            ot, lhsT=(v_sb[:, b * (S // 128) + kt, :]),
                            rhs=(et(kt)),
                            start=(kt == 0), stop=(kt == n_kt - 1),
                        )
                    nc.tensor.matmul(den, lhsT=ones_col, rhs=level[0],
                                     start=True, stop=True)
                    rd = wp2.tile([1, 256], MDT, name="rd", tag="rd")
                    nc.vector.reciprocal(rd, den)
                    rb = wp2.tile([128, 256], MDT, name="rb", tag="rb")
                    nc.gpsimd.partition_broadcast(rb, rd)
                    at = atp.tile([128, 256], MDT, name="at", tag="at")
                    nc.vector.tensor_mul(at, ot, rb)
                    return at

                def emit_oproj_part(b, qt, at_tiles, mq, split_dma=False,
                                    evict="mixed"):
                    """o-proj matmuls + eviction + store for one 128-row block.
                    evict: "mixed" = 1-2 ACT + rest DVE (balances ACT-paced
                    phase B); "act" = all ACT (keeps DVE clear near the end)."""
                    q0 = qt * 512
                    qh = mq // 2
                    mq2 = mq % 2
                    ob = op.tile([128, 4, 512], MDT, name="ob", tag="ob")
                    for nn in range(4):
                        po = psP.tile([128, 512], F32, name="po", tag="po")
                        for h in range(HQ):
                            nc.tensor.matmul(
                                po,
                                lhsT=(at_tiles[(h, qh)][:, mq2 * 128:(mq2 + 1) * 128]),
                                rhs=(wo_sb[:, h, nn * 512:(nn + 1) * 512]),
                                start=(h == 0), stop=(h == HQ - 1),
                            )
                        on_act = (nn % 2 == 0) if mq % 2 == 0 else (nn == 0)
                        if evict == "act" or on_act:
                            nc.scalar.copy(ob[:, nn, :], po)
                        else:
                            nc.vector.tensor_copy(ob[:, nn, :], po)
                        if split_dma and nn == 1:
                            nc.sync.dma_start(
                                out=out[b * S + q0 + mq * 128: b * S + q0 + (mq + 1) * 128,
                                        0:1024],
                                in_=ob[:, 0:2, :],
                            )
                    if split_dma:
                        nc.sync.dma_start(
                            out=out[b * S + q0 + mq * 128: b * S + q0 + (mq + 1) * 128,
                                    1024:2048],
                            in_=ob[:, 2:4, :],
                        )
                    else:
                        nc.sync.dma_start(
                            out=out[b * S + q0 + mq * 128: b * S + q0 + (mq + 1) * 128, :],
                            in_=ob,
                        )

                # chunk-level software pipeline: scores(c+1) issued before
                # den/pv(c); o-proj of the previous q-tile interleaved between
                # chunks to fill PE dependency bubbles. q-tiles visited
                # largest-first (3,0,1,2) so the pipeline starts with chunks
                # big enough to hide the exp latency.
                prev = None  # (b, qt, at_tiles)
                tiles = [(b, qt) for b in range(B) for qt in (3, 0, 1, 2)]
                for ti_, (b, qt) in enumerate(tiles):
                    last = ti_ == len(tiles) - 1
                    # qh0 chunks first so the drain's mq0/1 unblock early
                    chunks = [(h, qh) for qh in range(2) for h in range(HQ)]
                    at_tiles = {}
                    pend = []  # emitted scores awaiting den/pv
                    # small q-tiles have little PE work per chunk: run the
                    # scores->denpv pipeline two chunks deep to hide latency
                    depth = 1 if last else 2
                    for ci, (h, qh) in enumerate(chunks):
                        etps, qsums, n_kt = emit_scores(b, qt, h, qh)
                        if prev is not None:  # spread o-proj parts 1-per-chunk
                            pb_, pqt, pat = prev
                            emit_oproj_part(pb_, pqt, pat, ci,
                                            evict="act" if last else "mixed")
                        pend.append((h, qh, etps, qsums, n_kt))
                        if len(pend) > depth:
                            ph, pqh, petps, pq, pn = pend.pop(0)
                            at_tiles[(ph, pqh)] = emit_denpv(b, qt, ph, pqh, petps, pq, pn)
                            if last and ci == 2:  # qh0-row part needs only c0/c1
                                emit_oproj_part(b, qt, at_tiles, 0, evict="act")
                            if last and ci == 3:
                                emit_oproj_part(b, qt, at_tiles, 1, evict="act")
                    while pend:
                        ph, pqh, petps, pq, pn = pend.pop(0)
                        at_tiles[(ph, pqh)] = emit_denpv(b, qt, ph, pqh, petps, pq, pn)
                    prev = (b, qt, at_tiles)
                # drain the last q-tile's o-proj (mq0/1 already emitted above)
                pb_, pqt, pat = prev
                emit_oproj_part(pb_, pqt, pat, 2, evict="act")
                emit_oproj_part(pb_, pqt, pat, 3, split_dma=True, evict="act")
    nc.compile()
    return nc


def _rot_half(w):
    return np.concatenate([w[D // 2:], w[:D // 2]])


def prep_inputs(x, cos, sin, wq, wk, wv, wo, q_norm_w, k_norm_w):
    """Host-side sharding/layout prep. Returns per-core in_maps."""
    import ml_dtypes
    f = np.float32
    mf = np.dtype(ml_dtypes.bfloat16)
    cvt = lambda a: np.ascontiguousarray(a.astype(mf))
    x = np.asarray(x, f)
    cos = np.asarray(cos, f)
    sin = np.asarray(sin, f)
    wq, wk, wv, wo = (np.asarray(a, f) for a in (wq, wk, wv, wo))
    q_norm_w = np.asarray(q_norm_w, f)
    k_norm_w = np.asarray(k_norm_w, f)

    xt = np.ascontiguousarray(x.reshape(T, HID).T)  # [HID, T]
    ctq = np.ascontiguousarray(cos.T * q_norm_w[:, None])
    stq = np.ascontiguousarray(sin.T * _rot_half(q_norm_w)[:, None])
    ctk = np.ascontiguousarray(cos.T * k_norm_w[:, None])
    stk = np.ascontiguousarray(sin.T * _rot_half(k_norm_w)[:, None])
    # rotate-half permutation (with sign) as a matmul stationary operand:
    # out[d] = sum_j pmat[j, d] * q[j] = sign(d) * q[(d+64) % 128]
    pmat = np.zeros((D, D), f)
    for d in range(D // 2):
        pmat[d + D // 2, d] = -1.0
    for d in range(D // 2, D):
        pmat[d - D // 2, d] = 1.0
    onec = np.ones((D, 1), f)
    xt_m, ctq_m, stq_m, ctk_m, stk_m, pmat_m, onec_m = (
        cvt(a) for a in (xt, ctq, stq, ctk, stk, pmat, onec))

    in_maps = []
    for c in range(NCORES):
        wqkv_c = np.ascontiguousarray(np.concatenate([
            wq[:, c * HQ * D:(c + 1) * HQ * D],
            wk[:, c * D:(c + 1) * D],
            wv[:, c * D:(c + 1) * D],
        ], axis=1))
        woc = np.ascontiguousarray(wo[c * HQ * D:(c + 1) * HQ * D, :])
        in_maps.append({
            "xt": xt_m, "wqkv": cvt(wqkv_c), "woc": cvt(woc),
            "pmat": pmat_m, "onec": onec_m,
            "ctq": ctq_m, "stq": stq_m, "ctk": ctk_m, "stk": stk_m,
        })
    return in_maps


_NC = None


def get_nc():
    global _NC
    if _NC is None:
        _NC = build_nc()
    return _NC


def kernel(x, cos, sin, wq, wk, wv, wo, q_norm_w, k_norm_w):
    nc = get_nc()
    in_maps = prep_inputs(x, cos, sin, wq, wk, wv, wo, q_norm_w, k_norm_w)
    res = run_bass_kernel_spmd(nc, in_maps, core_ids=list(range(NCORES)))
    acc = np.zeros((T, HID), dtype=np.float64)
    for c in range(NCORES):
        acc += np.asarray(res.results[c]["out"], dtype=np.float64)
    return acc.astype(np.float32).reshape(B, S, HID)


# revision 46
# speedup vs baseline: 1.0005x; 1.0005x over previous
"""Trainium2 Bass kernel for a GQA attention block (B=2, S=2048, H=2048,
16 q-heads / 8 kv-heads, head_dim=128, fp32), tensor-parallel over heads
across 8 NeuronCores.

Per-core shard (core c): q-heads {2c, 2c+1}, kv-head c; wq/wk/wv column
shards, wo row shard. x is replicated (pre-transposed on host so the
contraction dim lands on SBUF partitions). Each core emits a bf16 partial
[4096, 2048] o-proj product; the host gather for the row-parallel o-proj
is a sum over the 8 partials.

All device data is bf16 (PSUM accumulation stays f32): halves HBM traffic
and enables the DVE 16-bit fast modes. DMAs are batched into whole-tile
transfers (the SP sequencer charges ~0.6-0.9us per DMA issue, so DMA
count matters as much as bytes).

Device dataflow (per core):
  A) Q^T/K^T projections in [d, tok] layout (f32 PSUM, ap-512 matmuls);
     V projected directly in natural [tok, d] layout (ap-128 matmuls,
     no PE transposes). RMSNorm sum-of-squares via GPSIMD partition
     all-reduce (q/k norm weights folded into the RoPE tables on host);
     RoPE rotate-half as a pmat matmul; rstd applied after RoPE.
  B) Causal attention per (batch, q-tile, head, 256-q-chunk):
     scores S^T [kpos,q] + exp on ACT (no max subtraction: RMSNorm
     bounds |scores| <= sqrt(128)) + causal affine_select on the
     diagonal pair; softmax denominator via bf16 pair/quad-tree adds on
     DVE + one ones-matmul per quad (PSUM-accumulated); PV with natural
     V stationary. The o-proj for q-tile T is interleaved with the
     attention chunks of q-tile T+1 to fill PE dependency bubbles.
"""

import math
import os
import sys

import numpy as np

for _p in ("/opt/trn_rl_repo", "/root/.axon_site/_ro/trn_rl_repo"):
    if os.path.isdir(_p) and _p not in sys.path:
        sys.path.insert(0, _p)
        break

import concourse.bacc as bacc
import concourse.tile as tile
from concourse import mybir

# Route every activation we use (Exp, Ln, Copy, Identity) to the single
# "natural_log_exp_and_others" table so the act-table-load pass emits one
# LoadActFuncSet for the whole kernel instead of thrashing between the
# exp-only and ln-only tables (1283ns per swap, on the ACT critical path).
# Only the pass's view of the *other* sets is altered; ids stay positional
# and the chosen set really does contain all four funcs.
_ORIG_GAT = bacc.get_activation_tables


def _gat_one_table(arch):
    AF = mybir.ActivationFunctionType
    tabs = {k: set(v) for k, v in _ORIG_GAT(arch).items()}
    strip = {AF.Exp, AF.Ln, AF.Copy, AF.Identity}
    for k in tabs:
        if k != "natural_log_exp_and_others":
            tabs[k] = tabs[k] - strip
    return tabs


bacc.get_activation_tables = _gat_one_table
from concourse.bass_isa import ReduceOp
from concourse.bass_utils import run_bass_kernel_spmd

# Problem constants (hardcoded per contract)
B, S, HID = 2, 2048, 2048
NH, NKV, D = 16, 8, 128
NCORES = 8
HQ = NH // NCORES  # q heads per core = 2
T = B * S          # 4096 tokens
EPS = 1e-5
F32 = mybir.dt.float32
BF16 = mybir.dt.bfloat16
MDT = BF16
SCALE = 1.0 / math.sqrt(D)

KT = HID // 128      # 16 contraction tiles
TT = T // 512        # 8 token tiles of 512
QT_PER_B = S // 512  # 4 q-tiles per batch


def build_nc():
    nc = bacc.Bacc("TRN2", target_bir_lowering=False, debug=False)
    xt = nc.dram_tensor("xt", [HID, T], MDT, kind="ExternalInput").ap()
    wqkv = nc.dram_tensor("wqkv", [HID, 4 * D], MDT, kind="ExternalInput").ap()
    woc = nc.dram_tensor("woc", [HQ * D, HID], MDT, kind="ExternalInput").ap()
    pmat = nc.dram_tensor("pmat", [D, D], MDT, kind="ExternalInput").ap()
    onec = nc.dram_tensor("onec", [D, 1], MDT, kind="ExternalInput").ap()
    ctq = nc.dram_tensor("ctq", [D, S], MDT, kind="ExternalInput").ap()
    stq = nc.dram_tensor("stq", [D, S], MDT, kind="ExternalInput").ap()
    ctk = nc.dram_tensor("ctk", [D, S], MDT, kind="ExternalInput").ap()
    stk = nc.dram_tensor("stk", [D, S], MDT, kind="ExternalInput").ap()
    out = nc.dram_tensor("out", [T, HID], MDT, kind="ExternalOutput").ap()

    xt_r = xt.rearrange("(kt p) t -> p kt t", p=128)
    wqkv_r = wqkv.rearrange("(kt p) m -> p kt m", p=128)

    with tile.TileContext(nc) as tc:
        from contextlib import ExitStack

        with ExitStack() as root:
            root.enter_context(nc.allow_low_precision(
                reason="bf16 device data validated against 2e-2 rel-err gate"))
            const = root.enter_context(tc.tile_pool(name="const", bufs=1))
            ones_col = const.tile([128, 1], MDT, name="ones_col")
            nc.scalar.dma_start(out=ones_col, in_=onec)
            pmat_sb = const.tile([D, D], MDT, name="pmat_sb")
            nc.scalar.dma_start(out=pmat_sb, in_=pmat)
            eps_col = const.tile([128, 1], F32, name="eps_col")
            nc.vector.memset(eps_col, EPS)

            res = root.enter_context(tc.tile_pool(name="res", bufs=1))
            wo_sb = res.tile([128, HQ, HID], MDT, name="wo_sb")
            qt_sb = res.tile([128, HQ, T], MDT, name="qt_sb")   # [d, h, tok]
            kt_sb = res.tile([128, T], MDT, name="kt_sb")       # [d, tok]
            v_sb = res.tile([128, T // 128, D], MDT, name="v_sb")  # [tok%128, chunk, d]
            tabs = {}
            for nm in ("cq", "sq", "ck", "sk"):
                tabs[nm] = res.tile([128, S], MDT, name="tab_" + nm)

            # ---------------- Phase A: QKV^T, norm, rope ----------------------
            with ExitStack() as pa:
                wqp = pa.enter_context(tc.tile_pool(name="wqp", bufs=1))
                xp = pa.enter_context(tc.tile_pool(name="xp", bufs=3))
                wp = pa.enter_context(tc.tile_pool(name="wp", bufs=2))
                psQ = pa.enter_context(tc.tile_pool(name="psQ", bufs=2, space="PSUM"))
                psK = pa.enter_context(tc.tile_pool(name="psK", bufs=2, space="PSUM"))
                psV = pa.enter_context(tc.tile_pool(name="psV", bufs=1, space="PSUM"))
                psR = pa.enter_context(tc.tile_pool(name="psR", bufs=1, space="PSUM"))

                wqkv_sb = wqp.tile([128, KT, 4 * D], MDT, name="wqkv_sb")

                tab_srcs = {"cq": ctq, "sq": stq, "ck": ctk, "sk": stk}
                for ti, t in enumerate((0, 4, 1, 5, 2, 6, 3, 7)):
                    xk = xp.tile([128, KT, 512], MDT, name="xk", tag="xk")
                    if ti == 0:
                        # interleave wqkv / x sub-DMAs so the first matmul can
                        # start after ~1.5us instead of waiting for full tiles
                        k0 = 0
                        for g, kw in enumerate((2, 2, 4, 4, 4)):
                            nc.sync.dma_start(
                                out=wqkv_sb[:, k0:k0 + kw, :],
                                in_=wqkv_r[:, k0:k0 + kw, :],
                            )
                            nc.sync.dma_start(
                                out=xk[:, k0:k0 + kw, :],
                                in_=xt_r[:, k0:k0 + kw, t * 512:(t + 1) * 512],
                            )
                            k0 += kw
                        # first table slices (s0=0): cheap, needed by ~15us
                        for nm in ("cq", "sq", "ck", "sk"):
                            nc.sync.dma_start(
                                out=tabs[nm][:, 0:512], in_=tab_srcs[nm][:, 0:512]
                            )
                    elif ti == 1:
                        # halves so tile-1 matmuls are not gated on one big DMA
                        for kh in range(2):
                            nc.sync.dma_start(
                                out=xk[:, 8 * kh:8 * (kh + 1), :],
                                in_=xt_r[:, 8 * kh:8 * (kh + 1), t * 512:(t + 1) * 512],
                            )
                    else:
                        nc.sync.dma_start(
                            out=xk, in_=xt_r[:, :, t * 512:(t + 1) * 512]
                        )
                        if ti == 2:  # table remainders
                            for nm in ("cq", "sq", "ck", "sk"):
                                nc.sync.dma_start(
                                    out=tabs[nm][:, 512:S], in_=tab_srcs[nm][:, 512:S]
                                )
                        if ti == 2:  # wo not needed until phase B
                            nc.sync.dma_start(
                                out=wo_sb, in_=woc.rearrange("(h p) n -> p h n", p=128)
                            )

                    # Q^T projections: [d, tok] layout, ap-512
                    q01 = psQ.tile([128, 2, 512], F32, name="q01", tag="q01")
                    for k in range(KT):
                        for m in range(2):
                            nc.tensor.matmul(
                                q01[:, m, :],
                                lhsT=(wqkv_sb[:, k, m * 128:(m + 1) * 128]),
                                rhs=(xk[:, k, :]),
                                start=(k == 0), stop=(k == KT - 1),
                            )
                    kps = psK.tile([128, 512], F32, name="kps", tag="kps")
                    for k in range(KT):
                        nc.tensor.matmul(
                            kps,
                            lhsT=(wqkv_sb[:, k, 2 * 128:3 * 128]),
                            rhs=(xk[:, k, :]),
                            start=(k == 0), stop=(k == KT - 1),
                        )
                    # V directly in natural [tok, d] layout, ap-128
                    vps = psV.tile([128, 4, 128], F32, name="vps", tag="vps")
                    for c in range(4):
                        for k in range(KT):
                            nc.tensor.matmul(
                                vps[:, c, :],
                                lhsT=(xk[:, k, c * 128:(c + 1) * 128]),
                                rhs=(wqkv_sb[:, k, 3 * 128:4 * 128]),
                                start=(k == 0), stop=(k == KT - 1),
                            )
                    nc.scalar.copy(v_sb[:, t * 4:(t + 1) * 4, :], vps)

                    s0 = (t % QT_PER_B) * 512  # position-in-sequence of this tile
                    for m, cosT, sinT in (
                        (0, tabs["cq"], tabs["sq"]),
                        (1, tabs["cq"], tabs["sq"]),
                        (2, tabs["ck"], tabs["sk"]),
                    ):
                        src = q01[:, m, :] if m < 2 else kps
                        qk = wp.tile([128, 512], MDT, name="qk", tag="qk")
                        nc.scalar.copy(qk, src)  # sole PSUM reader (ACT)
                        sq = wp.tile([128, 512], MDT, name="sq", tag="sq")
                        nc.vector.tensor_mul(sq, qk, qk)
                        nc.gpsimd.partition_all_reduce(sq, sq, 128, ReduceOp.add)
                        # rstd = (ssq/D+eps)^-0.5 as exp(-0.5*ln(.)): keeps every
                        # ACT func in one table (exp+ln+copy) => no
                        # LoadActFuncSet thrash when the scheduler interleaves
                        # phase-A chains with phase-B exps. The all-reduce
                        # already wrote ssq to every partition, so compute on
                        # the full tile and skip the partition broadcast.
                        rln = wp.tile([128, 512], F32, name="rln", tag="rln")
                        nc.scalar.activation(
                            rln, sq, mybir.ActivationFunctionType.Ln,
                            bias=eps_col, scale=1.0 / D,
                        )
                        rstd = wp.tile([128, 512], MDT, name="rstd", tag="rstd")
                        nc.scalar.activation(
                            rstd, rln, mybir.ActivationFunctionType.Exp, scale=-0.5,
                        )
                        shf = psR.tile([128, 512], F32, name="shf", tag="shf")
                        nc.tensor.matmul(shf, lhsT=pmat_sb, rhs=qk, start=True, stop=True)
                        t0 = wp.tile([128, 512], MDT, name="t0", tag="t0")
                        nc.vector.tensor_mul(t0, qk, cosT[:, s0:s0 + 512])
                        t1 = wp.tile([128, 512], MDT, name="t1", tag="t1")
                        nc.vector.tensor_mul(t1, shf, sinT[:, s0:s0 + 512])
                        tr = wp.tile([128, 512], MDT, name="tr", tag="tr")
                        nc.vector.tensor_add(tr, t0, t1)
                        if m < 2:
                            dst = qt_sb[:, m, t * 512:(t + 1) * 512]
                        else:
                            dst = kt_sb[:, t * 512:(t + 1) * 512]
                        nc.vector.tensor_mul(dst, tr, rstd)

            # ---------------- Phase B: causal attention + o-proj --------------
            with ExitStack() as pb:
                ep = pb.enter_context(tc.tile_pool(name="ep", bufs=12))
                wp2 = pb.enter_context(tc.tile_pool(name="wp2", bufs=4))
                sump = pb.enter_context(tc.tile_pool(name="sump", bufs=14))
                atp = pb.enter_context(tc.tile_pool(name="atp", bufs=10))
                op = pb.enter_context(tc.tile_pool(name="op", bufs=3))
                psS = pb.enter_context(tc.tile_pool(name="psS", bufs=2, space="PSUM"))
                psOD = pb.enter_context(tc.tile_pool(name="psOD", bufs=2, space="PSUM"))
                psP = pb.enter_context(tc.tile_pool(name="psP", bufs=2, space="PSUM"))

                def emit_scores(b, qt, h, qh):
                    """Sub-phase 1: score matmuls + exp + causal mask.
                    Exp is packed 4 k-tiles per instruction (amortizes the ACT
                    access latency; ACT paces phase B). Returns (etps, n_kt):
                    etps[i][:, j, :] holds k-tile 4*i+j."""
                    q0 = qt * 512
                    qq0 = q0 + qh * 256
                    n_kt = (qq0 + 256) // 128
                    etps = []
                    qsums = []
                    for kq in range((n_kt + 3) // 4):
                        nj = min(4, n_kt - 4 * kq)
                        st = psS.tile([128, 4, 256], F32, name="st", tag="st")
                        for j in range(nj):
                            kt = 4 * kq + j
                            nc.tensor.matmul(
                                st[:, j, :],
                                lhsT=(kt_sb[:, b * S + kt * 128: b * S + (kt + 1) * 128]),
                                rhs=(qt_sb[:, h, b * S + qq0: b * S + qq0 + 256]),
                                start=True, stop=True,
                            )
                        etp = ep.tile([128, 4, 256], MDT, name="et", tag="et")
                        nc.scalar.activation(
                            etp[:, 0:nj, :], st[:, 0:nj, :],
                            mybir.ActivationFunctionType.Exp, scale=SCALE,
                        )
                        if 4 * kq + nj == n_kt:  # diagonal pair: causal mask
                            for kt in (n_kt - 2, n_kt - 1):
                                nc.gpsimd.affine_select(
                                    out=etp[:, kt % 4, :], in_=etp[:, kt % 4, :],
                                    pattern=[[1, 256]],
                                    channel_multiplier=-1,
                                    base=-(kt * 128 - qq0),
                                    compare_op=mybir.AluOpType.is_ge,
                                    fill=0.0,
                                )
                        etps.append(etp)
                    return etps, qsums, n_kt

                def emit_denpv(b, qt, h, qh, etps, qsums, n_kt):
                    """Sub-phase 2: den tree + PV accumulation + at scaling."""
                    def et(kt):
                        return etps[kt // 4][:, kt % 4, :]

                    # streaming binary-counter reduction: carry[l] holds the
                    # partial sum of 2^(l+1) k-tiles; <=3 live tiles at a time
                    n_pair = n_kt // 2
                    carry = {}
                    for kp in range(n_pair):
                        cur = sump.tile([128, 256], MDT, name="ts", tag="ts")
                        nc.vector.tensor_add(cur, et(2 * kp), et(2 * kp + 1))
                        l = 0
                        while l in carry:
                            nxt = sump.tile([128, 256], MDT, name="ts", tag="ts")
                            nc.vector.tensor_add(nxt, carry.pop(l), cur)
                            cur = nxt
                            l += 1
                        carry[l] = cur
                    lvls = sorted(carry)
                    cur = carry[lvls[0]]
                    for l in lvls[1:]:
                        nxt = sump.tile([128, 256], MDT, name="ts", tag="ts")
                        nc.vector.tensor_add(nxt, carry[l], cur)
                        cur = nxt
                    level = [cur]
                    od = psOD.tile([128, 2, 256], F32, name="od", tag="od")
                    ot = od[:, 0, :]
                    den = od[0:1, 1, :]
                    # PV first (needs only the exp tiles); the den matmul's
                    # quad partials were computed during the scores pipeline,
                    # so it issues right at PV end with no tree wait
                    for kt in range(n_kt):
                        nc.tensor.matmul(
                            ot, lhsT=(v_sb[:, b * (S // 128) + kt, :]),
                            rhs=(et(kt)),
                            start=(kt == 0), stop=(kt == n_kt - 1),
                        )
                    nc.tensor.matmul(den, lhsT=ones_col, rhs=level[0],
                                     start=True, stop=True)
                    rd = wp2.tile([1, 256], MDT, name="rd", tag="rd")
                    nc.vector.reciprocal(rd, den)
                    rb = wp2.tile([128, 256], MDT, name="rb", tag="rb")
                    nc.gpsimd.partition_broadcast(rb, rd)
                    at = atp.tile([128, 256], MDT, name="at", tag="at")
                    nc.vector.tensor_mul(at, ot, rb)
                    return at

                def emit_oproj_part(b, qt, at_tiles, mq, split_dma=False,
                                    evict="mixed"):
                    """o-proj matmuls + eviction + store for one 128-row block.
                    evict: "mixed" = 1-2 ACT + rest DVE (balances ACT-paced
                    phase B); "act" = all ACT (keeps DVE clear near the end)."""
                    q0 = qt * 512
                    qh = mq // 2
                    mq2 = mq % 2
                    ob = op.tile([128, 4, 512], MDT, name="ob", tag="ob")
                    for nn in range(4):
                        po = psP.tile([128, 512], F32, name="po", tag="po")
                        for h in range(HQ):
                            nc.tensor.matmul(
                                po,
                                lhsT=(at_tiles[(h, qh)][:, mq2 * 128:(mq2 + 1) * 128]),
                                rhs=(wo_sb[:, h, nn * 512:(nn + 1) * 512]),
                                start=(h == 0), stop=(h == HQ - 1),
                            )
                        on_act = (nn % 2 == 0) if mq % 2 == 0 else (nn == 0)
                        if evict == "act" or on_act:
                            nc.scalar.copy(ob[:, nn, :], po)
                        else:
                            nc.vector.tensor_copy(ob[:, nn, :], po)
                        if split_dma and nn == 1:
                            nc.sync.dma_start(
                                out=out[b * S + q0 + mq * 128: b * S + q0 + (mq + 1) * 128,
                                        0:1024],
                                in_=ob[:, 0:2, :],
                            )
                    if split_dma:
                        nc.sync.dma_start(
                            out=out[b * S + q0 + mq * 128: b * S + q0 + (mq + 1) * 128,
                                    1024:2048],
                            in_=ob[:, 2:4, :],
                        )
                    else:
                        nc.sync.dma_start(
                            out=out[b * S + q0 + mq * 128: b * S + q0 + (mq + 1) * 128, :],
                            in_=ob,
                        )

                # chunk-level software pipeline: scores(c+1) issued before
                # den/pv(c); o-proj of the previous q-tile interleaved between
                # chunks to fill PE dependency bubbles. q-tiles visited
                # largest-first (3,0,1,2) so the pipeline starts with chunks
                # big enough to hide the exp latency.
                prev = None  # (b, qt, at_tiles)
                tiles = [(b, qt) for b in range(B) for qt in (3, 0, 1, 2)]
                for ti_, (b, qt) in enumerate(tiles):
                    last = ti_ == len(tiles) - 1
                    # qh0 chunks first so the drain's mq0/1 unblock early
                    chunks = [(h, qh) for qh in range(2) for h in range(HQ)]
                    at_tiles = {}
                    pend = []  # emitted scores awaiting den/pv
                    # small q-tiles have little PE work per chunk: run the
                    # scores->denpv pipeline two chunks deep to hide latency
                    depth = 1 if last else 2
                    for ci, (h, qh) in enumerate(chunks):
                        etps, qsums, n_kt = emit_scores(b, qt, h, qh)
                        if prev is not None:  # spread o-proj parts 1-per-chunk
                            pb_, pqt, pat = prev
                            emit_oproj_part(pb_, pqt, pat, ci,
                                            evict="act" if last else "mixed")
                        pend.append((h, qh, etps, qsums, n_kt))
                        if len(pend) > depth:
                            ph, pqh, petps, pq, pn = pend.pop(0)
                            at_tiles[(ph, pqh)] = emit_denpv(b, qt, ph, pqh, petps, pq, pn)
                            if last and ci == 2:  # qh0-row part needs only c0/c1
                                emit_oproj_part(b, qt, at_tiles, 0, evict="act")
                            if last and ci == 3:
                                emit_oproj_part(b, qt, at_tiles, 1, evict="act")
                    while pend:
                        ph, pqh, petps, pq, pn = pend.pop(0)
                        at_tiles[(ph, pqh)] = emit_denpv(b, qt, ph, pqh, petps, pq, pn)
                    prev = (b, qt, at_tiles)
                # drain the last q-tile's o-proj (mq0/1 already emitted above)
                pb_, pqt, pat = prev
                emit_oproj_part(pb_, pqt, pat, 2, evict="act")
                emit_oproj_part(pb_, pqt, pat, 3, split_dma=True, evict="act")
    nc.compile()
    return nc


def _rot_half(w):
    return np.concatenate([w[D // 2:], w[:D // 2]])


def prep_inputs(x, cos, sin, wq, wk, wv, wo, q_norm_w, k_norm_w):
    """Host-side sharding/layout prep. Returns per-core in_maps."""
    import ml_dtypes
    f = np.float32
    mf = np.dtype(ml_dtypes.bfloat16)
    cvt = lambda a: np.ascontiguousarray(a.astype(mf))
    x = np.asarray(x, f)
    cos = np.asarray(cos, f)
    sin = np.asarray(sin, f)
    wq, wk, wv, wo = (np.asarray(a, f) for a in (wq, wk, wv, wo))
    q_norm_w = np.asarray(q_norm_w, f)
    k_norm_w = np.asarray(k_norm_w, f)

    xt = np.ascontiguousarray(x.reshape(T, HID).T)  # [HID, T]
    ctq = np.ascontiguousarray(cos.T * q_norm_w[:, None])
    stq = np.ascontiguousarray(sin.T * _rot_half(q_norm_w)[:, None])
    ctk = np.ascontiguousarray(cos.T * k_norm_w[:, None])
    stk = np.ascontiguousarray(sin.T * _rot_half(k_norm_w)[:, None])
    # rotate-half permutation (with sign) as a matmul stationary operand:
    # out[d] = sum_j pmat[j, d] * q[j] = sign(d) * q[(d+64) % 128]
    pmat = np.zeros((D, D), f)
    for d in range(D // 2):
        pmat[d + D // 2, d] = -1.0
    for d in range(D // 2, D):
        pmat[d - D // 2, d] = 1.0
    onec = np.ones((D, 1), f)
    xt_m, ctq_m, stq_m, ctk_m, stk_m, pmat_m, onec_m = (
        cvt(a) for a in (xt, ctq, stq, ctk, stk, pmat, onec))

    in_maps = []
    for c in range(NCORES):
        wqkv_c = np.ascontiguousarray(np.concatenate([
            wq[:, c * HQ * D:(c + 1) * HQ * D],
            wk[:, c * D:(c + 1) * D],
            wv[:, c * D:(c + 1) * D],
        ], axis=1))
        woc = np.ascontiguousarray(wo[c * HQ * D:(c + 1) * HQ * D, :])
        in_maps.append({
            "xt": xt_m, "wqkv": cvt(wqkv_c), "woc": cvt(woc),
            "pmat": pmat_m, "onec": onec_m,
            "ctq": ctq_m, "stq": stq_m, "ctk": ctk_m, "stk": stk_m,
        })
    return in_maps


_NC = None


def get_nc():
    global _NC
    if _NC is None:
        _NC = build_nc()
    return _NC


def kernel(x, cos, sin, wq, wk, wv, wo, q_norm_w, k_norm_w):
    nc = get_nc()
    in_maps = prep_inputs(x, cos, sin, wq, wk, wv, wo, q_norm_w, k_norm_w)
    res = run_bass_kernel_spmd(nc, in_maps, core_ids=list(range(NCORES)))
    acc = np.zeros((T, HID), dtype=np.float64)
    for c in range(NCORES):
        acc += np.asarray(res.results[c]["out"], dtype=np.float64)
    return acc.astype(np.float32).reshape(B, S, HID)
